# revision 2
# baseline (speedup 1.0000x reference)
"""FiLM + per-sample block-diagonal expansion, data-parallel over 8 TRN2 cores.

Problem (hardcoded shapes):
  x_cond    [64, 1024] f32
  x_to_film [64, 1024, 128] f32
  W         [1024, 256] f32, b [256] f32
  out       [64, 1024, 1024] f32, block-diagonal per sample:
            out[s, k*128+r, k*128+c] = film[s, k*128+r, c], zeros elsewhere,
            where film = (1 + gamma[:,None,:]) * x_to_film + beta[:,None,:],
            [gamma|beta] = x_cond @ W + b.

Strategy: pure data parallel — 8 batch samples per core. The device computes
the Linear (on TensorE) and the FiLM modulation (ScalarE/VectorE per-partition
scale+bias with D on partitions), streaming x_to_film through SBUF. The
block-diagonal scatter of the (mostly-zero) 256 MB output is done during
host-side unsharding: the device returns the dense 4 MB FiLM result per core
and the host places the 128x128 diagonal blocks into a zeroed output.

Host-side layout prep: x_cond is fed transposed ([IN, BPC]) and x_to_film is
fed transposed per sample ([BPC, D, S]) so every DMA is contiguous and the
FiLM scale/bias are per-partition scalars.
"""

import numpy as np

B, S, D, IN, BLOCKS = 64, 1024, 128, 1024, 8
N_CORES = 8
BPC = B // N_CORES  # batch samples per core
KC = IN // 128      # contraction chunks

_CACHE = {}


def _build_nc(reps=1):
    from contextlib import ExitStack

    import concourse.tile as tile
    from concourse import bacc, mybir

    dt = mybir.dt.float32
    nc = bacc.Bacc(
        "TRN2", target_bir_lowering=False, debug=False, num_devices=N_CORES
    )

    x_condT = nc.dram_tensor("x_condT", [IN, BPC], dt, kind="ExternalInput").ap()
    x_filmT = nc.dram_tensor("x_filmT", [BPC, D, S], dt, kind="ExternalInput").ap()
    w_in = nc.dram_tensor("w_in", [IN, 2 * D], dt, kind="ExternalInput").ap()
    b_in = nc.dram_tensor("b_in", [2 * D], dt, kind="ExternalInput").ap()
    filmT = nc.dram_tensor("filmT", [BPC, D, S], dt, kind="ExternalOutput").ap()

    with tile.TileContext(nc) as tc:
        with ExitStack() as ctx:
            _body(ctx, tc, mybir, dt, x_condT, x_filmT, w_in, b_in, filmT, reps)
    nc.compile()
    return nc


def _body(ctx, tc, mybir, dt, x_condT, x_filmT, w_in, b_in, filmT, reps):
    nc = tc.nc

    const_pool = ctx.enter_context(tc.tile_pool(name="const", bufs=1))
    gb_pool = ctx.enter_context(tc.tile_pool(name="gb", bufs=1))
    psum_pool = ctx.enter_context(tc.tile_pool(name="psum", bufs=1, space="PSUM"))
    xf_pool = ctx.enter_context(tc.tile_pool(name="xf", bufs=4))
    out_pool = ctx.enter_context(tc.tile_pool(name="out", bufs=4))

    # Weights / cond / bias loads (contiguous chunks).
    w_sb = const_pool.tile([128, KC * 2 * D], dt)
    for c in range(KC):
        nc.sync.dma_start(
            w_sb[:, c * 256 : (c + 1) * 256], w_in[c * 128 : (c + 1) * 128, :]
        )
    xct_sb = const_pool.tile([128, KC * BPC], dt)
    for c in range(KC):
        nc.sync.dma_start(
            xct_sb[:, c * BPC : (c + 1) * BPC], x_condT[c * 128 : (c + 1) * 128, :]
        )
    b_sb = const_pool.tile([1, 2 * D], dt)
    nc.sync.dma_start(b_sb[0:1, :], b_in.rearrange("(p n) -> p n", p=1))
    ones_sb = const_pool.tile([1, BPC], dt)
    nc.vector.memset(ones_sb[0:1, :], 1.0)

    # gammaT/betaT [D, BPC] = W.T @ x_cond.T + b ⊗ ones  (no transposes needed)
    pg = psum_pool.tile([128, BPC], dt, tag="pg")
    pb = psum_pool.tile([128, BPC], dt, tag="pb")
    for c in range(KC):
        nc.tensor.matmul(
            pg[:, :],
            lhsT=w_sb[:, c * 256 : c * 256 + 128],
            rhs=xct_sb[:, c * BPC : (c + 1) * BPC],
            start=(c == 0),
            stop=False,
        )
    nc.tensor.matmul(
        pg[:, :], lhsT=b_sb[0:1, 0:128], rhs=ones_sb[0:1, :], start=False, stop=True
    )
    for c in range(KC):
        nc.tensor.matmul(
            pb[:, :],
            lhsT=w_sb[:, c * 256 + 128 : (c + 1) * 256],
            rhs=xct_sb[:, c * BPC : (c + 1) * BPC],
            start=(c == 0),
            stop=False,
        )
    nc.tensor.matmul(
        pb[:, :], lhsT=b_sb[0:1, 128:256], rhs=ones_sb[0:1, :], start=False, stop=True
    )

    gT = gb_pool.tile([128, BPC], dt, tag="gT")
    bT = gb_pool.tile([128, BPC], dt, tag="bT")
    nc.vector.tensor_scalar_add(gT[:, :], pg[:, :], 1.0)  # 1 + gamma
    nc.vector.tensor_copy(bT[:, :], pb[:, :])

    # FiLM stream: per sample, one [128, S] tile; out = gamma' * x + beta
    # (per-partition scale+bias). Alternate ScalarE / VectorE to halve the
    # elementwise critical path.
    for _ in range(reps):
        for s in range(BPC):
            xf = xf_pool.tile([128, S], dt, tag="xf")
            nc.sync.dma_start(xf[:, :], x_filmT[s])
            ot = out_pool.tile([128, S], dt, tag="ot")
            if s % 2 == 0:
                nc.scalar.activation(
                    ot[:, :],
                    xf[:, :],
                    mybir.ActivationFunctionType.Identity,
                    bias=bT[:, s : s + 1],
                    scale=gT[:, s : s + 1],
                )
            else:
                nc.vector.tensor_scalar(
                    ot[:, :],
                    xf[:, :],
                    gT[:, s : s + 1],
                    bT[:, s : s + 1],
                    op0=mybir.AluOpType.mult,
                    op1=mybir.AluOpType.add,
                )
            nc.sync.dma_start(filmT[s], ot[:, :])


def _get_nc(reps=1):
    key = ("nc", reps)
    if key not in _CACHE:
        _CACHE[key] = _build_nc(reps)
    return _CACHE[key]


def _make_in_maps(x_cond, x_to_film, W, b):
    in_maps = []
    for i in range(N_CORES):
        sl = slice(i * BPC, (i + 1) * BPC)
        in_maps.append(
            {
                "x_condT": np.ascontiguousarray(x_cond[sl].T),
                "x_filmT": np.ascontiguousarray(x_to_film[sl].transpose(0, 2, 1)),
                "w_in": np.ascontiguousarray(W),
                "b_in": np.ascontiguousarray(b),
            }
        )
    return in_maps


def _assemble(film_shards):
    # film_shards: list of [BPC, D, S] per core -> full [B, S, S] block-diag.
    filmT = np.concatenate(film_shards, axis=0)  # [B, D, S]
    film = filmT.transpose(0, 2, 1)  # [B, S, D]
    out = np.zeros((B, S, BLOCKS * D), dtype=np.float32)
    chunks = film.reshape(B, BLOCKS, S // BLOCKS, D)
    for k in range(BLOCKS):
        out[:, k * 128 : (k + 1) * 128, k * 128 : (k + 1) * 128] = chunks[:, k]
    return out[:, :, :S]


def kernel(x_cond, x_to_film, W, b):
    from concourse.bass_utils import run_bass_kernel_spmd

    nc = _get_nc()
    in_maps = _make_in_maps(
        np.asarray(x_cond, dtype=np.float32),
        np.asarray(x_to_film, dtype=np.float32),
        np.asarray(W, dtype=np.float32),
        np.asarray(b, dtype=np.float32),
    )
    res = run_bass_kernel_spmd(nc, in_maps, list(range(N_CORES)))
    return _assemble([r["filmT"] for r in res.results])
